# revision 5
# baseline (speedup 1.0000x reference)
"""Trainium2 Bass kernel for nn_ConvSelfAttention (conv_in -> agent-aware
attention -> conv_out), 3 dispatches:

  d1a: conv_in, seq-sharded (48 frames/core, all 2560 output channels in
       256-channel weight chunks -> every matmul tile is a full 128 wide).
  d1b: attention, head-sharded. Vector/gpsimd-free formulation: softmax
       normalization and agent-mask are folded into a second Exp activation
       (exp(s - ln z + mask*(-30))), and the same/other combine happens in
       PSUM via transpose-matmul accumulation.
  d2:  conv_out, seq-sharded (48 frames/core).

All matmuls f32r. Host does the reshards between dispatches (pure data/head
parallel; no cross-device communication inside any dispatch).
"""

import sys

sys.path.insert(0, "/opt/trn_rl_repo")

import numpy as np

import concourse.bacc as bacc
import concourse.tile as tile
import concourse.mybir as mybir
from concourse.bass_utils import run_bass_kernel_spmd

dt = mybir.dt

N_CORES = 8
SEQ = 384
C = 512
H = W = 8
HWP = 64          # pixels per image
NH = 8            # heads
HD = 64           # head dim
EMB = 5
SCALE = 1.0 / 8.0
NEG = -30.0       # exp(-30) ~ 9e-14: masked-out score contribution

F32 = dt.float32
F32R = dt.float32r
BF16 = dt.bfloat16

NQ = SEQ // N_CORES              # 48 frames/core for d1a+d2
CO_CHUNK = 256
N_CHUNK = (EMB * C) // CO_CHUNK  # 10
QG = 8                           # frames per psum group (8*64 = 512 free)
N_QG = NQ // QG                  # 6


# ---------------- dispatch builders ----------------

def build_d1a(repeat=1):
    """conv_in: xpad [4,128,48,100] f32r, w1 [10,128,4,9,256] f32r,
    b1 [128,20] f32 -> feats1 [10,128,2,6,512] f32."""
    nc = bacc.Bacc("TRN2", target_bir_lowering=False, debug=False,
                   num_devices=N_CORES)
    xpad = nc.dram_tensor("xpad", [4, 128, NQ, 100], F32R,
                          kind="ExternalInput").ap()
    w1 = nc.dram_tensor("w1", [N_CHUNK, 128, 4, 9, CO_CHUNK], F32R,
                        kind="ExternalInput").ap()
    b1 = nc.dram_tensor("b1", [128, 2 * N_CHUNK], F32,
                        kind="ExternalInput").ap()
    feats1 = nc.dram_tensor("feats1", [N_CHUNK, 128, 2, N_QG, 512], F32,
                            kind="ExternalOutput").ap()

    from contextlib import ExitStack

    with tile.TileContext(nc) as tc, ExitStack() as ctx:
        consts = ctx.enter_context(tc.tile_pool(name="consts", bufs=1))
        wpool = ctx.enter_context(tc.tile_pool(name="wpool", bufs=2))
        opool = ctx.enter_context(tc.tile_pool(name="opool", bufs=2))
        cps = ctx.enter_context(tc.tile_pool(name="cps", bufs=4, space="PSUM"))

        b_sb = consts.tile([128, 2 * N_CHUNK], F32, name="b_sb")
        nc.sync.dma_start(b_sb[:], b1)
        slab = consts.tile([128, 4, NQ, 100], F32R, name="slab")
        for cit in range(4):
            nc.scalar.dma_start(slab[:, cit], xpad[cit])

        for _rep in range(repeat):
            for ch in range(N_CHUNK):
                w_sb = wpool.tile([128, 4, 9, CO_CHUNK], F32R, tag="w")
                nc.sync.dma_start(w_sb[:], w1[ch])
                osb = opool.tile([128, 2, N_QG, 512], F32, tag="osb")
                for cot in range(2):
                    co0 = cot * 128
                    for qg in range(N_QG):
                        q0 = qg * QG
                        ps = cps.tile([128, QG, 8, 8], F32, tag="cps")
                        k = 0
                        for cit in range(4):
                            for tap in range(9):
                                ddy, ddx = tap // 3 - 1, tap % 3 - 1
                                rhs = slab[:, cit, q0:q0 + QG] \
                                    .rearrange("c q (y x) -> c q y x", y=10) \
                                    [:, :, 1 + ddy:9 + ddy, 1 + ddx:9 + ddx]
                                nc.tensor.matmul(
                                    ps[:], w_sb[:, cit, tap, co0:co0 + 128],
                                    rhs, start=(k == 0), stop=(k == 35))
                                k += 1
                        nc.scalar.activation(
                            osb[:, cot, qg].rearrange("c (q y x) -> c q y x",
                                                      q=QG, y=8),
                            ps[:], mybir.ActivationFunctionType.Identity,
                            bias=b_sb[:, 2 * ch + cot:2 * ch + cot + 1])
                nc.sync.dma_start(feats1[ch], osb[:])
    nc.compile()
    return nc


def build_d1b(repeat=1):
    """Attention for one head, vector-engine-free.

    Inputs (per core):
      F    [64, 3, 128, 384] f32r - [pixel, jt, dd, q]; jt0=[ks|ko],
           jt1=[qs|qo] (pre-scaled), jt2=[v|junk]
      am   [3, 128, 384] f32r - attn_mask q-tile major
      mskS [3, 128, 384] f32r - NEG*(1-m): kills same-stream where m=0
      mskO [3, 128, 384] f32r - NEG*m
      ident [128, 128] f32r
    Output:
      att [64, 64, 384] f32 - [pixel, d, q]
    """
    G = 8
    nc = bacc.Bacc("TRN2", target_bir_lowering=False, debug=False,
                   num_devices=N_CORES)
    F = nc.dram_tensor("F", [HWP, 3, 128, SEQ], F32R, kind="ExternalInput").ap()
    am = nc.dram_tensor("am", [3, 128, SEQ], F32R, kind="ExternalInput").ap()
    mskS = nc.dram_tensor("mskS", [3, 128, SEQ], F32R, kind="ExternalInput").ap()
    mskO = nc.dram_tensor("mskO", [3, 128, SEQ], F32R, kind="ExternalInput").ap()
    ident = nc.dram_tensor("ident", [128, 128], F32R, kind="ExternalInput").ap()
    att = nc.dram_tensor("att", [HWP, HD, SEQ], F32, kind="ExternalOutput").ap()

    Exp = mybir.ActivationFunctionType.Exp
    Ln = mybir.ActivationFunctionType.Ln
    Rcp = mybir.ActivationFunctionType.Reciprocal

    from contextlib import ExitStack

    with tile.TileContext(nc) as tc, ExitStack() as ctx:
        P = {}
        for name, bufs, space in [
                ("consts", 1, "SBUF"), ("fin", 2, "SBUF"), ("scr", 4, "SBUF"),
                ("xs", 4, "SBUF"), ("atT", 4, "SBUF"), ("vsb", 4, "SBUF"),
                ("og", 2, "SBUF"), ("zz", 4, "SBUF"),
                ("sps", 2, "PSUM"), ("tps", 1, "PSUM"),
                ("avps", 1, "PSUM")]:
            P[name] = ctx.enter_context(
                tc.tile_pool(name=name, bufs=bufs, space=space))

        id_sb = P["consts"].tile([128, 128], F32R, name="id_sb")
        nc.sync.dma_start(id_sb[:], ident)
        am_sb = P["consts"].tile([128, 3, SEQ], F32R, name="am_sb")
        mskS_sb = P["consts"].tile([128, 3, SEQ], F32R, name="mskS_sb")
        mskO_sb = P["consts"].tile([128, 3, SEQ], F32R, name="mskO_sb")
        for qt in range(3):
            nc.sync.dma_start(am_sb[:, qt], am[qt])
            nc.sync.dma_start(mskS_sb[:, qt], mskS[qt])
            nc.sync.dma_start(mskO_sb[:, qt], mskO[qt])
        msks = [mskS_sb, mskO_sb]

        def one_pixel(ft, og, pg):
            zsb = P["zz"].tile([128, 2, 3], F32, tag="zsb")
            rz = P["zz"].tile([128, 2, 3], F32, tag="rz")
            nlz = P["zz"].tile([128, 2, 3], F32, tag="nlz")
            xs = P["xs"].tile([128, 2, 3, SEQ], F32R, tag="xs")
            ks, qs, v = ft[:, 0], ft[:, 1], ft[0:64, 2]
            spss = []
            for so in range(2):
                qq = qs[64 * so:64 * so + 64]
                kk = ks[64 * so:64 * so + 64]
                sps = P["sps"].tile([128, 3, 512], F32, tag="sps")
                spss.append(sps)
                for qt in range(3):
                    nc.tensor.matmul(sps[:, qt, :SEQ], id_sb, am_sb[:, qt],
                                     start=True, stop=False)
                    nc.tensor.matmul(sps[:, qt, :SEQ],
                                     qq[:, qt * 128:(qt + 1) * 128], kk,
                                     start=False, stop=True)
                    scr = P["scr"].tile([128, SEQ], BF16, tag="scr")
                    nc.scalar.activation(scr[:], sps[:, qt, :SEQ], Exp,
                                         accum_out=zsb[:, so, qt:qt + 1])
                    # reopen the group to add the agent-mask pattern
                    nc.tensor.matmul(sps[:, qt, :SEQ], id_sb,
                                     msks[so][:, qt], start=False, stop=True,
                                     skip_group_check=True)
            # -ln(z) via ACT: lnz = Ln(z), then negate with scale=-1
            nc.scalar.activation(rz[:], zsb[:], Ln)
            nc.scalar.activation(nlz[:], rz[:],
                                 mybir.ActivationFunctionType.Identity,
                                 scale=-1.0)
            for so in range(2):
                for qt in range(3):
                    # exp(s + msk - ln z) = masked softmax row
                    nc.scalar.activation(xs[:, so, qt], spss[so][:, qt, :SEQ],
                                         Exp, bias=nlz[:, so, qt:qt + 1])

            # attn^T via transpose-matmuls, same+other accumulated in PSUM
            atT = P["atT"].tile([128, 3, SEQ], F32R, tag="atT")
            for kt in range(3):
                tps = P["tps"].tile([128, 512], F32R, tag="tps")
                for qt in range(3):
                    nc.tensor.matmul(
                        tps[:, qt * 128:(qt + 1) * 128],
                        xs[:, 0, qt, kt * 128:(kt + 1) * 128], id_sb,
                        is_transpose=True, start=True, stop=False)
                    nc.tensor.matmul(
                        tps[:, qt * 128:(qt + 1) * 128],
                        xs[:, 1, qt, kt * 128:(kt + 1) * 128], id_sb,
                        is_transpose=True, start=False, stop=True)
                nc.scalar.copy(atT[:, kt], tps[:, :SEQ])

            vps = P["tps"].tile([128, 3, HD], F32R, tag="tps", name="vps")
            for kt in range(3):
                nc.tensor.transpose(vps[:, kt], v[:, kt * 128:(kt + 1) * 128],
                                    id_sb[0:64, 0:64])
            vsb = P["vsb"].tile([128, 3, HD], F32R, tag="vsb")
            nc.scalar.copy(vsb[:], vps[:])

            avps = P["avps"].tile([HD, 512], F32, tag="avps")
            for kt in range(3):
                nc.tensor.matmul(avps[:, :SEQ], vsb[:, kt], atT[:, kt],
                                 start=(kt == 0), stop=(kt == 2))
            nc.scalar.copy(og[:, pg], avps[:, :SEQ])

        for _rep in range(repeat):
            for g0 in range(0, HWP, G):
                ftg = P["fin"].tile([128, G, 3, SEQ], F32R, tag="ftg")
                nc.sync.dma_start(
                    ftg[:], F[g0:g0 + G].rearrange("g j c q -> c g j q"))
                og = P["og"].tile([HD, G, SEQ], F32, tag="og", name="og")
                for pg in range(G):
                    one_pixel(ftg[:, pg], og, pg)
                nc.sync.dma_start(
                    att[g0:g0 + G].rearrange("g d q -> d g q"), og[:])
    nc.compile()
    return nc


def build_d2(repeat=1):
    """conv_out: x2 [4,128,48,100] f32r, w2 [128,4,4,9,128] f32r,
    b2 [128,4] f32 -> o2 [4,128,48,64] f32 ([cot, co, q, (y x)])."""
    nc = bacc.Bacc("TRN2", target_bir_lowering=False, debug=False,
                   num_devices=N_CORES)
    x2 = nc.dram_tensor("x2", [4, 128, NQ, 100], F32R, kind="ExternalInput").ap()
    w2 = nc.dram_tensor("w2", [128, 4, 4, 9, 128], F32R, kind="ExternalInput").ap()
    b2 = nc.dram_tensor("b2", [128, 4], F32, kind="ExternalInput").ap()
    o2 = nc.dram_tensor("o2", [4, 128, NQ, 64], F32, kind="ExternalOutput").ap()

    from contextlib import ExitStack

    with tile.TileContext(nc) as tc, ExitStack() as ctx:
        consts = ctx.enter_context(tc.tile_pool(name="consts", bufs=1))
        osbp = ctx.enter_context(tc.tile_pool(name="osbp", bufs=2))
        cps = ctx.enter_context(tc.tile_pool(name="cps", bufs=4, space="PSUM"))

        w_sb = consts.tile([128, 4, 4, 9, 128], F32R, name="w_sb")
        for cit in range(4):
            nc.scalar.dma_start(w_sb[:, cit], w2[:, cit])
        b_sb = consts.tile([128, 4], F32, name="b_sb")
        nc.sync.dma_start(b_sb[:], b2)
        slab = consts.tile([128, 4, NQ, 100], F32R, name="slab")
        for cit in range(4):
            nc.sync.dma_start(slab[:, cit], x2[cit])

        for _rep in range(repeat):
            for cot in range(4):
                osb = osbp.tile([128, N_QG, 512], F32, tag="osb")
                for qg in range(N_QG):
                    q0 = qg * QG
                    ps = cps.tile([128, QG, 8, 8], F32, tag="cps")
                    k = 0
                    for cit in range(4):
                        for tap in range(9):
                            ddy, ddx = tap // 3 - 1, tap % 3 - 1
                            rhs = slab[:, cit, q0:q0 + QG] \
                                .rearrange("c q (y x) -> c q y x", y=10) \
                                [:, :, 1 + ddy:9 + ddy, 1 + ddx:9 + ddx]
                            nc.tensor.matmul(
                                ps[:], w_sb[:, cit, cot, tap], rhs,
                                start=(k == 0), stop=(k == 35))
                            k += 1
                    nc.scalar.activation(
                        osb[:, qg].rearrange("c (q s) -> c q s", q=QG),
                        ps[:].rearrange("c q y x -> c q (y x)"),
                        mybir.ActivationFunctionType.Identity,
                        bias=b_sb[:, cot:cot + 1])
                nc.sync.dma_start(
                    o2[cot].rearrange("c q s -> c (q s)"),
                    osb[:].rearrange("c g q -> c (g q)"))
    nc.compile()
    return nc


# ---------------- host-side prep / reshard ----------------

def prep_d1a_inputs(inp, w_in, b_in):
    """Per-core xpad + shared w1/b1 (q channels pre-scaled by 1/sqrt(HD))."""
    w_s = w_in.astype(np.float32).copy()
    b_s = b_in.astype(np.float32).copy()
    idx = np.arange(EMB * C)
    qmask = (idx // NH >= 2 * HD) & (idx // NH < 4 * HD)
    w_s[qmask] *= SCALE
    b_s[qmask] *= SCALE
    w1 = np.ascontiguousarray(
        w_s.reshape(N_CHUNK, CO_CHUNK, 4, 128, 9)
        .transpose(0, 3, 2, 4, 1)).astype(np.float32)
    b1 = np.ascontiguousarray(
        b_s.reshape(N_CHUNK, 2, 128).transpose(2, 0, 1)
        .reshape(128, 2 * N_CHUNK)).astype(np.float32)

    maps = []
    for j in range(N_CORES):
        x = inp[0, j * NQ:(j + 1) * NQ]                  # [48, 512, 8, 8]
        xt = np.ascontiguousarray(x.transpose(1, 0, 2, 3))
        xp = np.zeros((C, NQ, 10, 10), dtype=np.float32)
        xp[:, :, 1:9, 1:9] = xt
        xpad = np.ascontiguousarray(
            xp.reshape(4, 128, NQ, 100)).astype(np.float32)
        maps.append({"xpad": xpad, "w1": w1, "b1": b1})
    return maps


def reshard_feats(res1, attn_mask, agent_aware_mask):
    """feats1 cores -> per-head d1b inputs."""
    parts = []
    for j in range(N_CORES):
        f = res1[j]["feats1"]                     # [10, 128, 2, 6, 512] f32
        f = f.reshape(N_CHUNK, 128, 2, N_QG, QG, HWP)
        f = f.transpose(0, 2, 1, 3, 4, 5).reshape(EMB * C, NQ, HWP)
        parts.append(f)
    O = np.concatenate(parts, axis=1)             # [2560, 384, 64] f32
    O = O.reshape(EMB * HD, NH, SEQ, HWP)         # [i, h, q, p]

    ident = np.eye(128, dtype=np.float32)
    maps = []
    for h in range(NH):
        Fh = np.zeros((HWP, 3, 128, SEQ), dtype=np.float32)
        kq = O[:256, h]                           # [256, 384, 64]
        Fh[:, :2] = kq.transpose(2, 0, 1).reshape(HWP, 2, 128, SEQ)
        v = O[256:320, h]                         # [64, 384, 64]
        Fh[:, 2, 0:64] = v.transpose(2, 0, 1)
        amh = np.ascontiguousarray(
            attn_mask[h].reshape(3, 128, SEQ)).astype(np.float32)
        mh = agent_aware_mask[h].astype(np.float32).reshape(3, 128, SEQ)
        mskS = np.ascontiguousarray(NEG * (1.0 - mh)).astype(np.float32)
        mskO = np.ascontiguousarray(NEG * mh).astype(np.float32)
        maps.append({"F": Fh, "am": amh, "mskS": mskS, "mskO": mskO,
                     "ident": ident})
    return maps


def reshard_att(res2, w_out, b_out):
    """att heads -> per-core d2 inputs."""
    A = np.zeros((HD, NH, SEQ, 10, 10), dtype=np.float32)  # [d, h, q, 10, 10]
    for h in range(NH):
        a = res2[h]["att"]                        # [64p, 64d, 384q] f32
        a = a.reshape(8, 8, HD, SEQ)              # [y, x, d, q]
        A[:, h, :, 1:9, 1:9] = a.transpose(2, 3, 0, 1)
    A = A.reshape(C, SEQ, 100)                    # channel c2 = d*8 + h

    w2 = np.ascontiguousarray(
        w_out.reshape(4, 128, 4, 128, 9)
        .transpose(3, 2, 0, 4, 1)).astype(np.float32)
    b2 = np.ascontiguousarray(
        b_out.reshape(4, 128).T).astype(np.float32)

    maps = []
    for j in range(N_CORES):
        x2 = np.ascontiguousarray(
            A[:, j * NQ:(j + 1) * NQ].reshape(4, 128, NQ, 100))
        maps.append({"x2": x2, "w2": w2, "b2": b2})
    return maps


def assemble_out(res3, b, seq, c):
    """o2 cores -> full output [b, seq, c, h, w]."""
    out = np.empty((seq, c, H, W), dtype=np.float32)
    for j in range(N_CORES):
        o = res3[j]["o2"]                         # [4, 128, 48, 64]
        out[j * NQ:(j + 1) * NQ] = (
            o.reshape(c, NQ, H, W).transpose(1, 0, 2, 3))
    return out.reshape(b, seq, c, H, W)


_NC_CACHE = {}


def _get_nc(name, builder, **kw):
    key = (name, tuple(sorted(kw.items())))
    if key not in _NC_CACHE:
        _NC_CACHE[key] = builder(**kw)
    return _NC_CACHE[key]


def kernel(inp, attn_mask, agent_aware_mask, w_in, b_in, w_out, b_out):
    inp = np.asarray(inp, dtype=np.float32)
    attn_mask = np.asarray(attn_mask, dtype=np.float32)
    agent_aware_mask = np.asarray(agent_aware_mask)
    w_in = np.asarray(w_in, dtype=np.float32)
    b_in = np.asarray(b_in, dtype=np.float32)
    w_out = np.asarray(w_out, dtype=np.float32)
    b_out = np.asarray(b_out, dtype=np.float32)

    b, seq, c, h, w = inp.shape
    assert (b, seq, c, h, w) == (1, SEQ, C, H, W)

    cores = list(range(N_CORES))
    nc1 = _get_nc("d1a", build_d1a)
    maps1 = prep_d1a_inputs(inp, w_in, b_in)
    res1 = run_bass_kernel_spmd(nc1, maps1, core_ids=cores).results

    nc2 = _get_nc("d1b", build_d1b)
    maps2 = reshard_feats(res1, attn_mask, agent_aware_mask)
    res2 = run_bass_kernel_spmd(nc2, maps2, core_ids=cores).results

    nc3 = _get_nc("d2", build_d2)
    maps3 = reshard_att(res2, w_out, b_out)
    res3 = run_bass_kernel_spmd(nc3, maps3, core_ids=cores).results

    return assemble_out(res3, b, seq, c)


# revision 6
# speedup vs baseline: 2.3719x; 2.3719x over previous
"""Trainium2 Bass kernel for nn_ConvSelfAttention (conv_in -> agent-aware
attention -> conv_out), 3 dispatches:

  d1a: conv_in, seq-sharded (48 frames/core, all 2560 output channels in
       256-channel weight chunks -> every matmul tile is a full 128 wide).
  d1b: attention, head-sharded. Vector/gpsimd-free formulation: softmax
       normalization and agent-mask are folded into a second Exp activation
       (exp(s - ln z + mask*(-30))), and the same/other combine happens in
       PSUM via transpose-matmul accumulation.
  d2:  conv_out, seq-sharded (48 frames/core).

All matmuls f32r. Host does the reshards between dispatches (pure data/head
parallel; no cross-device communication inside any dispatch).
"""

import sys

sys.path.insert(0, "/opt/trn_rl_repo")

import numpy as np

import concourse.bacc as bacc
import concourse.tile as tile
import concourse.mybir as mybir
from concourse.bass_utils import run_bass_kernel_spmd

dt = mybir.dt

N_CORES = 8
SEQ = 384
C = 512
H = W = 8
HWP = 64          # pixels per image
NH = 8            # heads
HD = 64           # head dim
EMB = 5
SCALE = 1.0 / 8.0
NEG = -30.0       # exp(-30) ~ 9e-14: masked-out score contribution

F32 = dt.float32
F32R = dt.float32r
BF16 = dt.bfloat16

NQ = SEQ // N_CORES              # 48 frames/core for d1a+d2
CO_CHUNK = 256
N_CHUNK = (EMB * C) // CO_CHUNK  # 10
QG = 8                           # frames per psum group (8*64 = 512 free)
N_QG = NQ // QG                  # 6


# ---------------- dispatch builders ----------------

def build_d1a(repeat=1):
    """conv_in: xpad [4,128,48,100] f32r, w1 [10,128,4,9,256] f32r,
    b1 [128,20] f32 -> feats1 [10,128,2,6,512] f32."""
    nc = bacc.Bacc("TRN2", target_bir_lowering=False, debug=False,
                   num_devices=N_CORES)
    xpad = nc.dram_tensor("xpad", [4, 128, NQ, 100], F32R,
                          kind="ExternalInput").ap()
    w1 = nc.dram_tensor("w1", [N_CHUNK, 128, 4, 9, CO_CHUNK], F32R,
                        kind="ExternalInput").ap()
    b1 = nc.dram_tensor("b1", [128, 2 * N_CHUNK], F32,
                        kind="ExternalInput").ap()
    feats1 = nc.dram_tensor("feats1", [N_CHUNK, 128, 2, N_QG, 512], BF16,
                            kind="ExternalOutput").ap()

    from contextlib import ExitStack

    with tile.TileContext(nc) as tc, ExitStack() as ctx:
        consts = ctx.enter_context(tc.tile_pool(name="consts", bufs=1))
        wpool = ctx.enter_context(tc.tile_pool(name="wpool", bufs=2))
        opool = ctx.enter_context(tc.tile_pool(name="opool", bufs=2))
        cps = ctx.enter_context(tc.tile_pool(name="cps", bufs=4, space="PSUM"))

        b_sb = consts.tile([128, 2 * N_CHUNK], F32, name="b_sb")
        nc.sync.dma_start(b_sb[:], b1)
        slab = consts.tile([128, 4, NQ, 100], F32R, name="slab")
        for cit in range(4):
            nc.scalar.dma_start(slab[:, cit], xpad[cit])

        for _rep in range(repeat):
            for ch in range(N_CHUNK):
                w_sb = wpool.tile([128, 4, 9, CO_CHUNK], F32R, tag="w")
                nc.sync.dma_start(w_sb[:], w1[ch])
                osb = opool.tile([128, 2, N_QG, 512], BF16, tag="osb")
                for cot in range(2):
                    co0 = cot * 128
                    for qg in range(N_QG):
                        q0 = qg * QG
                        ps = cps.tile([128, QG, 8, 8], F32, tag="cps")
                        k = 0
                        for cit in range(4):
                            for tap in range(9):
                                ddy, ddx = tap // 3 - 1, tap % 3 - 1
                                rhs = slab[:, cit, q0:q0 + QG] \
                                    .rearrange("c q (y x) -> c q y x", y=10) \
                                    [:, :, 1 + ddy:9 + ddy, 1 + ddx:9 + ddx]
                                nc.tensor.matmul(
                                    ps[:], w_sb[:, cit, tap, co0:co0 + 128],
                                    rhs, start=(k == 0), stop=(k == 35))
                                k += 1
                        nc.scalar.activation(
                            osb[:, cot, qg].rearrange("c (q y x) -> c q y x",
                                                      q=QG, y=8),
                            ps[:], mybir.ActivationFunctionType.Identity,
                            bias=b_sb[:, 2 * ch + cot:2 * ch + cot + 1])
                nc.sync.dma_start(feats1[ch], osb[:])
    nc.compile()
    return nc


def build_d1b(repeat=1):
    """Attention for one head, vector-engine-free.

    Inputs (per core):
      F    [64, 3, 128, 384] f32r - [pixel, jt, dd, q]; jt0=[ks|ko],
           jt1=[qs|qo] (pre-scaled), jt2=[v|junk]
      am   [3, 128, 384] f32r - attn_mask q-tile major
      mskS [3, 128, 384] f32r - NEG*(1-m): kills same-stream where m=0
      mskO [3, 128, 384] f32r - NEG*m
      ident [128, 128] f32r
    Output:
      att [64, 64, 384] f32 - [pixel, d, q]
    """
    G = 8
    nc = bacc.Bacc("TRN2", target_bir_lowering=False, debug=False,
                   num_devices=N_CORES)
    F = nc.dram_tensor("F", [HWP, 3, 128, SEQ], F32R, kind="ExternalInput").ap()
    am = nc.dram_tensor("am", [3, 128, SEQ], F32R, kind="ExternalInput").ap()
    mskS = nc.dram_tensor("mskS", [3, 128, SEQ], F32R, kind="ExternalInput").ap()
    mskO = nc.dram_tensor("mskO", [3, 128, SEQ], F32R, kind="ExternalInput").ap()
    ident = nc.dram_tensor("ident", [128, 128], F32R, kind="ExternalInput").ap()
    att = nc.dram_tensor("att", [HWP, HD, SEQ], F32, kind="ExternalOutput").ap()

    Exp = mybir.ActivationFunctionType.Exp
    Ln = mybir.ActivationFunctionType.Ln
    Rcp = mybir.ActivationFunctionType.Reciprocal

    from contextlib import ExitStack

    with tile.TileContext(nc) as tc, ExitStack() as ctx:
        P = {}
        for name, bufs, space in [
                ("consts", 1, "SBUF"), ("fin", 2, "SBUF"), ("scr", 4, "SBUF"),
                ("xs", 4, "SBUF"), ("atT", 4, "SBUF"), ("vsb", 4, "SBUF"),
                ("og", 2, "SBUF"), ("zz", 4, "SBUF"),
                ("sps", 2, "PSUM"), ("tps", 1, "PSUM"),
                ("avps", 1, "PSUM")]:
            P[name] = ctx.enter_context(
                tc.tile_pool(name=name, bufs=bufs, space=space))

        id_sb = P["consts"].tile([128, 128], F32R, name="id_sb")
        nc.sync.dma_start(id_sb[:], ident)
        am_sb = P["consts"].tile([128, 3, SEQ], F32R, name="am_sb")
        mskS_sb = P["consts"].tile([128, 3, SEQ], F32R, name="mskS_sb")
        mskO_sb = P["consts"].tile([128, 3, SEQ], F32R, name="mskO_sb")
        for qt in range(3):
            nc.sync.dma_start(am_sb[:, qt], am[qt])
            nc.sync.dma_start(mskS_sb[:, qt], mskS[qt])
            nc.sync.dma_start(mskO_sb[:, qt], mskO[qt])
        msks = [mskS_sb, mskO_sb]

        def one_pixel(ft, og, pg):
            zsb = P["zz"].tile([128, 2, 3], F32, tag="zsb")
            rz = P["zz"].tile([128, 2, 3], F32, tag="rz")
            nlz = P["zz"].tile([128, 2, 3], F32, tag="nlz")
            xs = P["xs"].tile([128, 2, 3, SEQ], F32R, tag="xs")
            ks, qs, v = ft[:, 0], ft[:, 1], ft[0:64, 2]
            spss = []
            for so in range(2):
                qq = qs[64 * so:64 * so + 64]
                kk = ks[64 * so:64 * so + 64]
                sps = P["sps"].tile([128, 3, 512], F32, tag="sps")
                spss.append(sps)
                for qt in range(3):
                    nc.tensor.matmul(sps[:, qt, :SEQ], id_sb, am_sb[:, qt],
                                     start=True, stop=False)
                    nc.tensor.matmul(sps[:, qt, :SEQ],
                                     qq[:, qt * 128:(qt + 1) * 128], kk,
                                     start=False, stop=True)
                    scr = P["scr"].tile([128, SEQ], BF16, tag="scr")
                    nc.scalar.activation(scr[:], sps[:, qt, :SEQ], Exp,
                                         accum_out=zsb[:, so, qt:qt + 1])
                    # reopen the group to add the agent-mask pattern
                    nc.tensor.matmul(sps[:, qt, :SEQ], id_sb,
                                     msks[so][:, qt], start=False, stop=True,
                                     skip_group_check=True)
            # -ln(z) via ACT: lnz = Ln(z), then negate with scale=-1
            nc.scalar.activation(rz[:], zsb[:], Ln)
            nc.scalar.activation(nlz[:], rz[:],
                                 mybir.ActivationFunctionType.Identity,
                                 scale=-1.0)
            for so in range(2):
                for qt in range(3):
                    # exp(s + msk - ln z) = masked softmax row
                    nc.scalar.activation(xs[:, so, qt], spss[so][:, qt, :SEQ],
                                         Exp, bias=nlz[:, so, qt:qt + 1])

            # attn^T via transpose-matmuls, same+other accumulated in PSUM
            atT = P["atT"].tile([128, 3, SEQ], F32R, tag="atT")
            for kt in range(3):
                tps = P["tps"].tile([128, 512], F32R, tag="tps")
                for qt in range(3):
                    nc.tensor.matmul(
                        tps[:, qt * 128:(qt + 1) * 128],
                        xs[:, 0, qt, kt * 128:(kt + 1) * 128], id_sb,
                        is_transpose=True, start=True, stop=False)
                    nc.tensor.matmul(
                        tps[:, qt * 128:(qt + 1) * 128],
                        xs[:, 1, qt, kt * 128:(kt + 1) * 128], id_sb,
                        is_transpose=True, start=False, stop=True)
                nc.scalar.copy(atT[:, kt], tps[:, :SEQ])

            vps = P["tps"].tile([128, 3, HD], F32R, tag="tps", name="vps")
            for kt in range(3):
                nc.tensor.transpose(vps[:, kt], v[:, kt * 128:(kt + 1) * 128],
                                    id_sb[0:64, 0:64])
            vsb = P["vsb"].tile([128, 3, HD], F32R, tag="vsb")
            nc.scalar.copy(vsb[:], vps[:])

            avps = P["avps"].tile([HD, 512], F32, tag="avps")
            for kt in range(3):
                nc.tensor.matmul(avps[:, :SEQ], vsb[:, kt], atT[:, kt],
                                 start=(kt == 0), stop=(kt == 2))
            nc.scalar.copy(og[:, pg], avps[:, :SEQ])

        for _rep in range(repeat):
            for g0 in range(0, HWP, G):
                ftg = P["fin"].tile([128, G, 3, SEQ], F32R, tag="ftg")
                nc.sync.dma_start(
                    ftg[:], F[g0:g0 + G].rearrange("g j c q -> c g j q"))
                og = P["og"].tile([HD, G, SEQ], F32, tag="og", name="og")
                for pg in range(G):
                    one_pixel(ftg[:, pg], og, pg)
                nc.sync.dma_start(
                    att[g0:g0 + G].rearrange("g d q -> d g q"), og[:])
    nc.compile()
    return nc


def build_d2(repeat=1):
    """conv_out: x2 [4,128,48,100] f32r, w2 [128,4,4,9,128] f32r,
    b2 [128,4] f32 -> o2 [4,128,48,64] f32 ([cot, co, q, (y x)])."""
    nc = bacc.Bacc("TRN2", target_bir_lowering=False, debug=False,
                   num_devices=N_CORES)
    x2 = nc.dram_tensor("x2", [4, 128, NQ, 100], F32R, kind="ExternalInput").ap()
    w2 = nc.dram_tensor("w2", [128, 4, 4, 9, 128], F32R, kind="ExternalInput").ap()
    b2 = nc.dram_tensor("b2", [128, 4], F32, kind="ExternalInput").ap()
    o2 = nc.dram_tensor("o2", [4, 128, NQ, 64], F32, kind="ExternalOutput").ap()

    from contextlib import ExitStack

    with tile.TileContext(nc) as tc, ExitStack() as ctx:
        consts = ctx.enter_context(tc.tile_pool(name="consts", bufs=1))
        osbp = ctx.enter_context(tc.tile_pool(name="osbp", bufs=2))
        cps = ctx.enter_context(tc.tile_pool(name="cps", bufs=4, space="PSUM"))

        w_sb = consts.tile([128, 4, 4, 9, 128], F32R, name="w_sb")
        for cit in range(4):
            nc.scalar.dma_start(w_sb[:, cit], w2[:, cit])
        b_sb = consts.tile([128, 4], F32, name="b_sb")
        nc.sync.dma_start(b_sb[:], b2)
        slab = consts.tile([128, 4, NQ, 100], F32R, name="slab")
        for cit in range(4):
            nc.sync.dma_start(slab[:, cit], x2[cit])

        for _rep in range(repeat):
            for cot in range(4):
                osb = osbp.tile([128, N_QG, 512], F32, tag="osb")
                for qg in range(N_QG):
                    q0 = qg * QG
                    ps = cps.tile([128, QG, 8, 8], F32, tag="cps")
                    k = 0
                    for cit in range(4):
                        for tap in range(9):
                            ddy, ddx = tap // 3 - 1, tap % 3 - 1
                            rhs = slab[:, cit, q0:q0 + QG] \
                                .rearrange("c q (y x) -> c q y x", y=10) \
                                [:, :, 1 + ddy:9 + ddy, 1 + ddx:9 + ddx]
                            nc.tensor.matmul(
                                ps[:], w_sb[:, cit, cot, tap], rhs,
                                start=(k == 0), stop=(k == 35))
                            k += 1
                    nc.scalar.activation(
                        osb[:, qg].rearrange("c (q s) -> c q s", q=QG),
                        ps[:].rearrange("c q y x -> c q (y x)"),
                        mybir.ActivationFunctionType.Identity,
                        bias=b_sb[:, cot:cot + 1])
                nc.sync.dma_start(
                    o2[cot].rearrange("c q s -> c (q s)"),
                    osb[:].rearrange("c g q -> c (g q)"))
    nc.compile()
    return nc


# ---------------- host-side prep / reshard ----------------

def prep_d1a_inputs(inp, w_in, b_in):
    """Per-core xpad + shared w1/b1 (q channels pre-scaled by 1/sqrt(HD))."""
    w_s = w_in.astype(np.float32).copy()
    b_s = b_in.astype(np.float32).copy()
    idx = np.arange(EMB * C)
    qmask = (idx // NH >= 2 * HD) & (idx // NH < 4 * HD)
    w_s[qmask] *= SCALE
    b_s[qmask] *= SCALE
    w1 = np.ascontiguousarray(
        w_s.reshape(N_CHUNK, CO_CHUNK, 4, 128, 9)
        .transpose(0, 3, 2, 4, 1)).astype(np.float32)
    b1 = np.ascontiguousarray(
        b_s.reshape(N_CHUNK, 2, 128).transpose(2, 0, 1)
        .reshape(128, 2 * N_CHUNK)).astype(np.float32)

    maps = []
    for j in range(N_CORES):
        x = inp[0, j * NQ:(j + 1) * NQ]                  # [48, 512, 8, 8]
        xt = np.ascontiguousarray(x.transpose(1, 0, 2, 3))
        xp = np.zeros((C, NQ, 10, 10), dtype=np.float32)
        xp[:, :, 1:9, 1:9] = xt
        xpad = np.ascontiguousarray(
            xp.reshape(4, 128, NQ, 100)).astype(np.float32)
        maps.append({"xpad": xpad, "w1": w1, "b1": b1})
    return maps


def reshard_feats(res1, attn_mask, agent_aware_mask):
    """feats1 cores -> per-head d1b inputs."""
    parts = []
    for j in range(N_CORES):
        f = np.asarray(res1[j]["feats1"], dtype=np.float32)
        f = f.reshape(N_CHUNK, 128, 2, N_QG, QG, HWP)
        f = f.transpose(0, 2, 1, 3, 4, 5).reshape(EMB * C, NQ, HWP)
        parts.append(f)
    O = np.concatenate(parts, axis=1)             # [2560, 384, 64] f32
    O = O.reshape(EMB * HD, NH, SEQ, HWP)         # [i, h, q, p]

    ident = np.eye(128, dtype=np.float32)
    maps = []
    for h in range(NH):
        Fh = np.zeros((HWP, 3, 128, SEQ), dtype=np.float32)
        kq = O[:256, h]                           # [256, 384, 64]
        Fh[:, :2] = kq.transpose(2, 0, 1).reshape(HWP, 2, 128, SEQ)
        v = O[256:320, h]                         # [64, 384, 64]
        Fh[:, 2, 0:64] = v.transpose(2, 0, 1)
        amh = np.ascontiguousarray(
            attn_mask[h].reshape(3, 128, SEQ)).astype(np.float32)
        mh = agent_aware_mask[h].astype(np.float32).reshape(3, 128, SEQ)
        mskS = np.ascontiguousarray(NEG * (1.0 - mh)).astype(np.float32)
        mskO = np.ascontiguousarray(NEG * mh).astype(np.float32)
        maps.append({"F": Fh, "am": amh, "mskS": mskS, "mskO": mskO,
                     "ident": ident})
    return maps


def reshard_att(res2, w_out, b_out):
    """att heads -> per-core d2 inputs."""
    A = np.zeros((HD, NH, SEQ, 10, 10), dtype=np.float32)  # [d, h, q, 10, 10]
    for h in range(NH):
        a = res2[h]["att"]                        # [64p, 64d, 384q] f32
        a = a.reshape(8, 8, HD, SEQ)              # [y, x, d, q]
        A[:, h, :, 1:9, 1:9] = a.transpose(2, 3, 0, 1)
    A = A.reshape(C, SEQ, 100)                    # channel c2 = d*8 + h

    w2 = np.ascontiguousarray(
        w_out.reshape(4, 128, 4, 128, 9)
        .transpose(3, 2, 0, 4, 1)).astype(np.float32)
    b2 = np.ascontiguousarray(
        b_out.reshape(4, 128).T).astype(np.float32)

    maps = []
    for j in range(N_CORES):
        x2 = np.ascontiguousarray(
            A[:, j * NQ:(j + 1) * NQ].reshape(4, 128, NQ, 100))
        maps.append({"x2": x2, "w2": w2, "b2": b2})
    return maps


def assemble_out(res3, b, seq, c):
    """o2 cores -> full output [b, seq, c, h, w]."""
    out = np.empty((seq, c, H, W), dtype=np.float32)
    for j in range(N_CORES):
        o = res3[j]["o2"]                         # [4, 128, 48, 64]
        out[j * NQ:(j + 1) * NQ] = (
            o.reshape(c, NQ, H, W).transpose(1, 0, 2, 3))
    return out.reshape(b, seq, c, H, W)


_NC_CACHE = {}


def _get_nc(name, builder, **kw):
    key = (name, tuple(sorted(kw.items())))
    if key not in _NC_CACHE:
        _NC_CACHE[key] = builder(**kw)
    return _NC_CACHE[key]


def kernel(inp, attn_mask, agent_aware_mask, w_in, b_in, w_out, b_out):
    inp = np.asarray(inp, dtype=np.float32)
    attn_mask = np.asarray(attn_mask, dtype=np.float32)
    agent_aware_mask = np.asarray(agent_aware_mask)
    w_in = np.asarray(w_in, dtype=np.float32)
    b_in = np.asarray(b_in, dtype=np.float32)
    w_out = np.asarray(w_out, dtype=np.float32)
    b_out = np.asarray(b_out, dtype=np.float32)

    b, seq, c, h, w = inp.shape
    assert (b, seq, c, h, w) == (1, SEQ, C, H, W)

    cores = list(range(N_CORES))
    nc1 = _get_nc("d1a", build_d1a)
    maps1 = prep_d1a_inputs(inp, w_in, b_in)
    res1 = run_bass_kernel_spmd(nc1, maps1, core_ids=cores).results

    nc2 = _get_nc("d1b", build_d1b)
    maps2 = reshard_feats(res1, attn_mask, agent_aware_mask)
    res2 = run_bass_kernel_spmd(nc2, maps2, core_ids=cores).results

    nc3 = _get_nc("d2", build_d2)
    maps3 = reshard_att(res2, w_out, b_out)
    res3 = run_bass_kernel_spmd(nc3, maps3, core_ids=cores).results

    return assemble_out(res3, b, seq, c)
